# revision 15
# baseline (speedup 1.0000x reference)
"""Expert-parallel MoE (top-2 of 8 experts, SwiGLU) for 8 Trainium2 NeuronCores.

Sharding: expert-parallel. Core e holds expert e's weights (w_gate[e], w_up[e],
w_down[e]); x and the router weights are replicated. Each core (all SPMD, one
program):
  1. Router (replicated, exact fp32 on PE): logits = x @ w_router.T, top-2 via
     vector.max, softmax over the two selected logits.
  2. Selects its own expert's tokens (one-hot input per core), stream-compacts
     the token ids with a matmul-based prefix sum, and scatters (token-id,
     combine-weight) into per-slot arrays with indirect DMA.
  3. Gathers its tokens, transposes them on the PE, and runs the expert FFN in
     float32r (full-rate fp32 matmuls): gT/uT = W @ xgT, actT = silu(gT)*uT,
     yT = w_down @ actT, scaled by the per-token combine weight.
  4. Scatters the per-token results into a dense [T, H] partial output and
     ReduceScatters across the 8 cores; each core returns its [T/8, H] shard.

The host only shards inputs, picks the compile-time token capacity (from a
cheap numpy estimate of the same routing; the device routing is authoritative
and bounds-checked), and concatenates the 8 output shards.
"""

import math
import sys

import numpy as np

sys.path.insert(0, "/opt/trn_rl_repo")

from concourse import bacc, bass, mybir, tile  # noqa: E402
from concourse.bass import IndirectOffsetOnAxis  # noqa: E402
from concourse.bass_utils import run_bass_kernel_spmd  # noqa: E402
from concourse.masks import make_identity  # noqa: E402

F32 = mybir.dt.float32
F32R = mybir.dt.float32r
I32 = mybir.dt.int32
AF = mybir.ActivationFunctionType
ALU = mybir.AluOpType
AX = mybir.AxisListType

P = 128
NCORES = 8


def _c_chunks(c):
    """Split the token-slot dim into moving-operand chunks, each in [256, 512]
    (float32r runs at full rate only when the moving dim is >= 256)."""
    assert c % P == 0 and c >= 256
    out = []
    rem = c
    while rem > 512:
        take = 512 if rem - 512 >= 256 or rem == 512 else 384
        out.append(take)
        rem -= take
    if rem:
        if rem < 256 and out:
            out[-1] -= 256 - rem
            rem = 256
        out.append(rem)
    assert sum(out) == c and all(256 <= w <= 512 for w in out), (c, out)
    return out


def build_moe(T, H, I, E, CPAD, n_cores=NCORES):
    """Build the SPMD Bass program. Returns the compiled Bacc object."""
    HC = H // P  # h chunks (contraction dim of stage 1)
    IC = I // P  # i chunks (contraction dim of stage 2)
    TT = T // P  # token tiles
    CT = CPAD // P  # slot tiles
    chunks = _c_chunks(CPAD)
    coffs = [sum(chunks[:j]) for j in range(len(chunks))]
    psum_bufs = 2 if len(chunks) <= 2 else 1

    nc = bacc.Bacc(
        "TRN2", target_bir_lowering=False, debug=False, num_devices=n_cores
    )

    x_d = nc.dram_tensor("x", [T, H], F32, kind="ExternalInput").ap()
    wr_d = nc.dram_tensor("wr", [E, H], F32, kind="ExternalInput").ap()
    wg_d = nc.dram_tensor("wg", [H, I], F32R, kind="ExternalInput").ap()
    wu_d = nc.dram_tensor("wu", [H, I], F32R, kind="ExternalInput").ap()
    wd_d = nc.dram_tensor("wd", [I, H], F32R, kind="ExternalInput").ap()
    esel_d = nc.dram_tensor("esel", [P, E], F32, kind="ExternalInput").ap()
    out_d = nc.dram_tensor("out", [T // n_cores, H], F32, kind="ExternalOutput").ap()

    with tile.TileContext(nc) as tc:
        import contextlib

        with contextlib.ExitStack() as top:
            dram = top.enter_context(tc.tile_pool(name="dram", bufs=1, space="DRAM"))
            # slot arrays (+P rows of trash for padding slots)
            gidx_t = dram.tile([CPAD + P, 1], I32)  # gather idx, prefilled 0
            sidx_t = dram.tile([CPAD + P, 1], I32)  # scatter idx, prefilled T
            warr_t = dram.tile([CPAD + P, 1], F32)  # combine weight, prefilled 0
            part_t = dram.tile([T + P, H], F32)  # dense partial out (+trash row blk)
            rs_t = dram.tile([T // n_cores, H], F32)

            const = top.enter_context(tc.tile_pool(name="const", bufs=1))
            ident = const.tile([P, P], F32)
            make_identity(nc, ident)
            ones_col = const.tile([P, 1], F32)
            nc.vector.memset(ones_col, 1.0)
            # strict-lower-triangular-transposed masks: a[p, f] = 1 if f > p
            iot_f = const.tile([P, P], F32)
            nc.gpsimd.iota(
                iot_f, pattern=[[1, P]], channel_multiplier=0,
                allow_small_or_imprecise_dtypes=True,
            )
            iot_p = const.tile([P, 1], F32)
            nc.gpsimd.iota(
                iot_p, pattern=[[1, 1]], channel_multiplier=1,
                allow_small_or_imprecise_dtypes=True,
            )
            a128 = const.tile([P, P], F32)
            nc.vector.tensor_scalar(a128, iot_f, iot_p, None, op0=ALU.is_gt)
            a16 = const.tile([P, TT], F32)
            nc.vector.tensor_scalar(
                a16, iot_f[:, :TT], iot_p, None, op0=ALU.is_gt
            )
            tokid = const.tile([P, TT], I32)
            nc.gpsimd.iota(tokid, pattern=[[P, TT]], channel_multiplier=1)
            esel_s = const.tile([P, E], F32)
            nc.sync.dma_start(esel_s, esel_d)

            # router flags / weights / positions for this core's expert
            flags = const.tile([P, TT], F32)
            wvals = const.tile([P, TT], F32)

            # ---------------- phase A: router + compaction -------------------
            with contextlib.ExitStack() as ph:
                rp = ph.enter_context(tc.tile_pool(name="router", bufs=3))
                rps = ph.enter_context(
                    tc.tile_pool(name="router_ps", bufs=2, space="PSUM")
                )
                rps1 = ph.enter_context(
                    tc.tile_pool(name="router_ps1", bufs=1, space="PSUM")
                )
                zp = ph.enter_context(tc.tile_pool(name="zfill", bufs=1))

                # prefill slot arrays + zero the dense partial output
                zi = zp.tile([P, CT + 1], I32)
                nc.vector.memset(zi, 0)
                nc.gpsimd.dma_start(
                    gidx_t[:].rearrange("(f p) one -> p (f one)", p=P), zi
                )
                si = zp.tile([P, CT + 1], I32)
                nc.vector.memset(si, T)
                nc.gpsimd.dma_start(
                    sidx_t[:].rearrange("(f p) one -> p (f one)", p=P), si
                )
                zf = zp.tile([P, CT + 1], F32)
                nc.vector.memset(zf, 0.0)
                nc.gpsimd.dma_start(
                    warr_t[:].rearrange("(f p) one -> p (f one)", p=P), zf
                )
                zrow = zp.tile([P, H], F32)
                nc.vector.memset(zrow, 0.0)
                for r in range(TT):
                    nc.gpsimd.dma_start(part_t[r * P : (r + 1) * P, :], zrow)

                # w_router^T blocks [h, hc, e] via PE transpose of [E, H]
                wr_s = rp.tile([max(E, 8), H], F32, name="wr_nat")
                nc.sync.dma_start(wr_s[:E, :], wr_d)
                wrT = const.tile([P, HC, E], F32)
                for hc in range(HC):
                    tp = rps1.tile([P, E], F32, tag="wrt_ps")
                    nc.tensor.matmul(
                        tp,
                        lhsT=wr_s[:E, hc * P : (hc + 1) * P],
                        rhs=ident[:E, :E],
                        is_transpose=True,
                        start=True,
                        stop=True,
                    )
                    nc.vector.tensor_copy(wrT[:, hc, :], tp)

                for tt in range(TT):
                    xt = rp.tile([P, H], F32, tag="xrow")
                    nc.sync.dma_start(xt, x_d[tt * P : (tt + 1) * P, :])
                    xTb = rp.tile([P, HC, P], F32, tag="xTb")
                    for hc in range(HC):
                        tp = rps.tile([P, P], F32, tag="tp_ps")
                        nc.tensor.transpose(
                            tp, xt[:, hc * P : (hc + 1) * P], ident
                        )
                        nc.vector.tensor_copy(xTb[:, hc, :], tp)
                    lg_ps = rps1.tile([P, E], F32, tag="lg_ps")
                    for hc in range(HC):
                        nc.tensor.matmul(
                            lg_ps,
                            lhsT=xTb[:, hc, :],
                            rhs=wrT[:, hc, :],
                            start=(hc == 0),
                            stop=(hc == HC - 1),
                        )
                    lg = rp.tile([P, E], F32, tag="lg")
                    nc.vector.tensor_copy(lg, lg_ps)
                    top8 = rp.tile([P, 8], F32, tag="top8")
                    nc.vector.max(out=top8, in_=lg)
                    v1 = top8[:, 0:1]
                    v2 = top8[:, 1:2]
                    # softmax over the two selected logits
                    sm = rp.tile([P, 4], F32, tag="sm")
                    d21 = sm[:, 0:1]
                    e21 = sm[:, 1:2]
                    w1 = sm[:, 2:3]
                    w2 = sm[:, 3:4]
                    nc.vector.tensor_sub(d21, v2, v1)
                    nc.scalar.activation(e21, d21, AF.Exp)
                    nc.vector.tensor_scalar_add(w1, e21, 1.0)
                    nc.vector.reciprocal(w1, w1)
                    nc.vector.tensor_mul(w2, e21, w1)
                    # per-expert selection flags and combine weights
                    sel = rp.tile([P, E], F32, tag="sel")
                    nc.vector.tensor_scalar(sel, lg, v2, None, op0=ALU.is_ge)
                    eq1 = rp.tile([P, E], F32, tag="eq1")
                    nc.vector.tensor_scalar(eq1, lg, v1, None, op0=ALU.is_equal)
                    eq2 = rp.tile([P, E], F32, tag="eq2")
                    nc.vector.tensor_scalar(eq2, lg, v2, None, op0=ALU.is_equal)
                    nc.vector.tensor_scalar(eq1, eq1, w1, None, op0=ALU.mult)
                    nc.vector.tensor_scalar(eq2, eq2, w2, None, op0=ALU.mult)
                    wm = rp.tile([P, E], F32, tag="wm")
                    nc.vector.tensor_add(wm, eq1, eq2)
                    # this core's expert column (esel is a one-hot row)
                    nc.vector.tensor_mul(sel, sel, esel_s)
                    nc.vector.reduce_sum(flags[:, tt : tt + 1], sel, axis=AX.X)
                    nc.vector.tensor_mul(wm, wm, esel_s)
                    nc.vector.reduce_sum(wvals[:, tt : tt + 1], wm, axis=AX.X)

                # prefix sums -> slot positions
                cs_ps = rps1.tile([TT, 1], F32, tag="cs_ps")
                nc.tensor.matmul(
                    cs_ps, lhsT=flags, rhs=ones_col, start=True, stop=True
                )
                cs_pad = rp.tile([P, 1], F32, name="cs_pad")
                nc.vector.memset(cs_pad, 0.0)
                nc.vector.tensor_copy(cs_pad[:TT, :], cs_ps)
                cs_bc = rp.tile([P, P], F32, name="cs_bc")
                nc.vector.tensor_copy(cs_bc, cs_pad[:, 0:1].to_broadcast((P, P)))
                cb_ps = rps1.tile([P, TT], F32, tag="cb_ps")
                nc.tensor.matmul(
                    cb_ps, lhsT=cs_bc, rhs=a16, start=True, stop=True
                )
                ic_ps = rps1.tile([P, TT], F32, tag="ic_ps")
                nc.tensor.matmul(
                    ic_ps, lhsT=a128, rhs=flags, start=True, stop=True
                )
                cb_sb = rp.tile([P, TT], F32, name="cb_sb")
                nc.vector.tensor_copy(cb_sb, cb_ps)
                pos = rp.tile([P, TT], F32, name="pos")
                nc.vector.tensor_add(pos, ic_ps, cb_sb)
                flags_i = rp.tile([P, TT], I32, name="flags_i")
                nc.vector.tensor_copy(flags_i, flags)
                posm = rp.tile([P, TT], F32, name="posm")
                nc.vector.memset(posm, float(CPAD))
                nc.vector.copy_predicated(posm, flags_i, pos)
                posmi = rp.tile([P, TT], I32, name="posmi")
                nc.vector.tensor_copy(posmi, posm)

                for tt in range(TT):
                    off = IndirectOffsetOnAxis(ap=posmi[:, tt : tt + 1], axis=0)
                    nc.gpsimd.indirect_dma_start(
                        out=gidx_t[:],
                        out_offset=off,
                        in_=tokid[:, tt : tt + 1],
                        in_offset=None,
                        bounds_check=CPAD + P - 1,
                        oob_is_err=False,
                    )
                    nc.gpsimd.indirect_dma_start(
                        out=sidx_t[:],
                        out_offset=off,
                        in_=tokid[:, tt : tt + 1],
                        in_offset=None,
                        bounds_check=CPAD + P - 1,
                        oob_is_err=False,
                    )
                    nc.gpsimd.indirect_dma_start(
                        out=warr_t[:],
                        out_offset=off,
                        in_=wvals[:, tt : tt + 1],
                        in_offset=None,
                        bounds_check=CPAD + P - 1,
                        oob_is_err=False,
                    )

            # ---------------- phase B: gather + stage 1 ----------------------
            act_pool = top.enter_context(tc.tile_pool(name="actp", bufs=1))
            actT = act_pool.tile([P, IC, CPAD], F32R)

            with contextlib.ExitStack() as ph:
                xgT_pool = ph.enter_context(tc.tile_pool(name="xgTp", bufs=1))
                xgT = xgT_pool.tile([P, HC, CPAD], F32R)
                with contextlib.ExitStack() as gph:
                    gxp = gph.enter_context(tc.tile_pool(name="gxp", bufs=2))
                    gps = gph.enter_context(
                        tc.tile_pool(name="gps", bufs=4, space="PSUM")
                    )
                    for ct in range(CT):
                        gi = gxp.tile([P, 1], I32, tag="gi")
                        nc.gpsimd.dma_start(gi, gidx_t[ct * P : (ct + 1) * P, :])
                        xg = gxp.tile([P, H], F32, tag="xg")
                        nc.gpsimd.indirect_dma_start(
                            out=xg,
                            out_offset=None,
                            in_=x_d,
                            in_offset=IndirectOffsetOnAxis(ap=gi[:, 0:1], axis=0),
                        )
                        for hc in range(HC):
                            tp = gps.tile([P, P], F32, tag="gtp")
                            nc.tensor.transpose(
                                tp, xg[:, hc * P : (hc + 1) * P], ident
                            )
                            nc.vector.tensor_copy(
                                xgT[:, hc, ct * P : (ct + 1) * P], tp
                            )

                w1p = ph.enter_context(tc.tile_pool(name="w1p", bufs=2))
                s1ps = ph.enter_context(
                    tc.tile_pool(name="s1ps", bufs=psum_bufs, space="PSUM")
                )
                for ic in range(IC):
                    wgt = w1p.tile([P, HC, P], F32R, tag="wg")
                    nc.sync.dma_start(
                        wgt,
                        wg_d[:, ic * P : (ic + 1) * P].rearrange(
                            "(hc p) i -> p hc i", p=P
                        ),
                    )
                    wut = w1p.tile([P, HC, P], F32R, tag="wu")
                    nc.sync.dma_start(
                        wut,
                        wu_d[:, ic * P : (ic + 1) * P].rearrange(
                            "(hc p) i -> p hc i", p=P
                        ),
                    )
                    pgs = [
                        s1ps.tile([P, cw], F32, tag=f"pg{j}", name=f"pg{j}_{ic}")
                        for j, cw in enumerate(chunks)
                    ]
                    pus = [
                        s1ps.tile([P, cw], F32, tag=f"pu{j}", name=f"pu{j}_{ic}")
                        for j, cw in enumerate(chunks)
                    ]
                    for hc in range(HC):
                        lg_ = wgt[:, hc, :]
                        for j, (c0, cw) in enumerate(zip(coffs, chunks)):
                            nc.tensor.matmul(
                                pgs[j],
                                lhsT=lg_,
                                rhs=xgT[:, hc, c0 : c0 + cw],
                                start=(hc == 0),
                                stop=(hc == HC - 1),
                            )
                        lu_ = wut[:, hc, :]
                        for j, (c0, cw) in enumerate(zip(coffs, chunks)):
                            nc.tensor.matmul(
                                pus[j],
                                lhsT=lu_,
                                rhs=xgT[:, hc, c0 : c0 + cw],
                                start=(hc == 0),
                                stop=(hc == HC - 1),
                            )
                    for j, (c0, cw) in enumerate(zip(coffs, chunks)):
                        # silu(g)*u = g*sigmoid(g)*u (sim lacks Silu)
                        nc.scalar.activation(
                            actT[:, ic, c0 : c0 + cw], pgs[j], AF.Sigmoid
                        )
                        nc.vector.tensor_mul(
                            actT[:, ic, c0 : c0 + cw],
                            actT[:, ic, c0 : c0 + cw],
                            pgs[j],
                        )
                        nc.vector.tensor_mul(
                            actT[:, ic, c0 : c0 + cw],
                            actT[:, ic, c0 : c0 + cw],
                            pus[j],
                        )

            # ---------------- phase C: stage 2 + combine ---------------------
            with contextlib.ExitStack() as ph:
                w2p = ph.enter_context(tc.tile_pool(name="w2p", bufs=2))
                wcp = ph.enter_context(tc.tile_pool(name="wcp", bufs=1))
                wcols = wcp.tile([P, CT], F32)
                nc.sync.dma_start(
                    wcols, warr_t[0 : CPAD, :].rearrange("(f p) one -> p f", p=P)
                )
                s2ps = ph.enter_context(
                    tc.tile_pool(name="s2ps", bufs=psum_bufs, space="PSUM")
                )
                t2ps = ph.enter_context(
                    tc.tile_pool(name="t2ps", bufs=2, space="PSUM")
                )
                yp = ph.enter_context(tc.tile_pool(name="yp", bufs=2))
                ybig = ph.enter_context(tc.tile_pool(name="ybig", bufs=1))
                ycts = [ybig.tile([P, H], F32, name=f"yct{ct}") for ct in range(CT)]

                ICH = IC // 2  # half-panels of w_down for double buffering
                for hc in range(HC):
                    wds = []
                    for half in range(2):
                        wdt = w2p.tile([P, ICH, P], F32R, tag="wd")
                        nc.sync.dma_start(
                            wdt,
                            wd_d[
                                half * ICH * P : (half + 1) * ICH * P,
                                hc * P : (hc + 1) * P,
                            ].rearrange("(ic p) h -> p ic h", p=P),
                        )
                        wds.append(wdt)
                    pys = [
                        s2ps.tile([P, cw], F32, tag=f"py{j}", name=f"py{j}_{hc}")
                        for j, cw in enumerate(chunks)
                    ]
                    for ic in range(IC):
                        ld = wds[ic // ICH][:, ic % ICH, :]
                        for j, (c0, cw) in enumerate(zip(coffs, chunks)):
                            nc.tensor.matmul(
                                pys[j],
                                lhsT=ld,
                                rhs=actT[:, ic, c0 : c0 + cw],
                                start=(ic == 0),
                                stop=(ic == IC - 1),
                            )
                    yts = yp.tile([P, CPAD], F32, tag="yts")
                    for j, (c0, cw) in enumerate(zip(coffs, chunks)):
                        nc.vector.tensor_copy(yts[:, c0 : c0 + cw], pys[j])
                    for ct in range(CT):
                        tp = t2ps.tile([P, P], F32, tag="ytp")
                        nc.tensor.transpose(
                            tp, yts[:, ct * P : (ct + 1) * P], ident
                        )
                        nc.vector.tensor_scalar(
                            ycts[ct][:, hc * P : (hc + 1) * P],
                            tp,
                            wcols[:, ct : ct + 1],
                            None,
                            op0=ALU.mult,
                        )

                sxp = ph.enter_context(tc.tile_pool(name="sxp", bufs=2))
                for ct in range(CT):
                    si_ = sxp.tile([P, 1], I32, tag="si")
                    nc.gpsimd.dma_start(si_, sidx_t[ct * P : (ct + 1) * P, :])
                    nc.gpsimd.indirect_dma_start(
                        out=part_t[:],
                        out_offset=IndirectOffsetOnAxis(ap=si_[:, 0:1], axis=0),
                        in_=ycts[ct],
                        in_offset=None,
                    )

            nc.gpsimd.collective_compute(
                "ReduceScatter",
                ALU.add,
                replica_groups=[list(range(n_cores))],
                ins=[part_t[0:T, :].opt()],
                outs=[rs_t[:].opt()],
            )
            nc.sync.dma_start(out_d, rs_t[:])

    nc.compile()
    return nc


# ---------------------------------------------------------------------------

_CACHE = {}

T0, H0, I0, E0 = 2048, 2048, 5632, 8


def _capacity(x, w_router, top_k):
    logits = x.astype(np.float32) @ w_router.astype(np.float32).T
    k = int(top_k)
    idx = np.argpartition(-logits, k - 1, axis=-1)[:, :k]
    counts = np.bincount(idx.ravel(), minlength=w_router.shape[0])
    cmax = int(counts.max())
    return max(256, P * math.ceil((cmax + 16) / P))


def kernel(x, w_router, w_gate, w_up, w_down, top_k, _trace=False):
    x = np.ascontiguousarray(np.asarray(x, dtype=np.float32))
    w_router = np.ascontiguousarray(np.asarray(w_router, dtype=np.float32))
    w_gate = np.asarray(w_gate, dtype=np.float32)
    w_up = np.asarray(w_up, dtype=np.float32)
    w_down = np.asarray(w_down, dtype=np.float32)
    assert int(top_k) == 2, f"kernel specialized for top_k=2, got {top_k}"
    T, H = x.shape
    E, I = w_gate.shape[0], w_gate.shape[1]
    assert (T, H, I, E) == (T0, H0, I0, E0), "kernel hardcoded for spec shapes"

    cpad = _capacity(x, w_router, top_k)
    if cpad not in _CACHE:
        _CACHE[cpad] = build_moe(T, H, I, E, cpad)
    nc = _CACHE[cpad]

    eye = np.eye(E, dtype=np.float32)
    in_maps = [
        {
            "x": x,
            "wr": w_router,
            "wg": np.ascontiguousarray(w_gate[e].T),
            "wu": np.ascontiguousarray(w_up[e].T),
            "wd": np.ascontiguousarray(w_down[e].T),
            "esel": np.repeat(eye[e : e + 1], P, axis=0),
        }
        for e in range(NCORES)
    ]
    import time as _time

    t0 = _time.time()
    res = run_bass_kernel_spmd(
        nc, in_maps, core_ids=list(range(NCORES)), trace=False
    )
    kernel._last_wall_s = _time.time() - t0
    kernel._last_exec_time_ns = res.exec_time_ns
    out = np.concatenate([res.results[c]["out"] for c in range(NCORES)], axis=0)
    return out
